# revision 2
# baseline (speedup 1.0000x reference)
"""Trainium2 Bass kernel for nn_CheriBlock (dilated conv + global norm + MLP + residual).

Per-sample computation (reference):
    conv = w0*x[l-d] + w1*x[l] + w2*x[l+d]          (depthwise, zero-padded, d=8)
    x_conv = (conv - mean) * rstd                    (mean/var over whole [L,C] slab)
    h = gelu_tanh(x_conv @ W1.T)                     ([L, 2C])
    out = X + (h @ W2.T) * gamma

Sharding: data-parallel over N (8 samples -> 8 cores). Weights replicated.

Device-side algebra:
  - Normalization is deferred past MM1 (linearity):
        rstd*(conv - mean) @ W1T = rstd*(conv @ W1T) - rstd*mean*s1
    applied inside the gelu activation as per-partition scale/bias.
  - gamma is folded into W2 on the host.
  - Activations for MM1 need [C, L] layout: x is cast to bf16 into a DRAM
    bounce buffer, then DMA-transposed (xbar) into SBUF.
"""

import numpy as np

_CACHE = {}

P = 128
L = 8192
C = 512
H = 1024
D = 8            # dilation
NCB = C // P     # 4 c-blocks
NHB = H // P     # 8 h-blocks
CHUNK = 2048     # l-chunk for cast/transpose/conv
NCHUNK = L // CHUNK
LT = 512         # l-tile for the MM phase
NLT = L // LT
HALO = 16        # halo columns on each side of xt (16 for 32B DMA alignment)
N_CORES = 8


def _build_module():
    import concourse.bass as bass
    import concourse.bacc as bacc
    import concourse.tile as tile
    import concourse.mybir as mybir

    f32 = mybir.dt.float32
    bf16 = mybir.dt.bfloat16
    AF = mybir.ActivationFunctionType
    OP = mybir.AluOpType
    AX = mybir.AxisListType
    ts = bass.ts

    nc = bacc.Bacc("TRN2", target_bir_lowering=False, debug=False)

    x_d = nc.dram_tensor("x", [L, C], f32, kind="ExternalInput").ap()
    w1t_d = nc.dram_tensor("w1t", [C, H], bf16, kind="ExternalInput").ap()
    w2tg_d = nc.dram_tensor("w2tg", [H, C], bf16, kind="ExternalInput").ap()
    cwt_d = nc.dram_tensor("cwt", [C, 3], f32, kind="ExternalInput").ap()
    s1g_d = nc.dram_tensor("s1g", [P, NHB], f32, kind="ExternalInput").ap()
    ones_d = nc.dram_tensor("ones", [P, P], f32, kind="ExternalInput").ap()
    out_d = nc.dram_tensor("out", [L, C], f32, kind="ExternalOutput").ap()

    with tile.TileContext(nc) as tc:
        with (
            tc.tile_pool(name="const", bufs=1) as const,
            tc.tile_pool(name="dram", bufs=1, space="DRAM") as dram,
            tc.tile_pool(name="xtp", bufs=1) as xtp,
            tc.tile_pool(name="convp", bufs=1) as convp,
            tc.tile_pool(name="work", bufs=2) as work,
            tc.tile_pool(name="hp", bufs=2) as hp,
            tc.tile_pool(name="outp", bufs=2) as outp,
            tc.tile_pool(name="psum", bufs=1, space="PSUM") as psum,
        ):
            # ---- constants ----
            w1t_r = w1t_d.rearrange("(cb p) h -> cb p h", p=P)
            w1t_sb = []
            for cb in range(NCB):
                t = const.tile([P, H], bf16, name=f"w1t{cb}")
                nc.sync.dma_start(t[:], w1t_r[cb])
                w1t_sb.append(t)
            w2tg_r = w2tg_d.rearrange("(hb p) c -> hb p c", p=P)
            w2tg_sb = []
            for hb in range(NHB):
                t = const.tile([P, C], bf16, name=f"w2tg{hb}")
                nc.sync.dma_start(t[:], w2tg_r[hb])
                w2tg_sb.append(t)
            cwt_r = cwt_d.rearrange("(cb p) t -> cb p t", p=P)
            cw_sb = []
            for cb in range(NCB):
                t = const.tile([P, 3], f32, name=f"cw{cb}")
                nc.sync.dma_start(t[:], cwt_r[cb])
                cw_sb.append(t)
            s1g_sb = const.tile([P, NHB], f32, name="s1g_sb")
            nc.sync.dma_start(s1g_sb[:], s1g_d[:])
            ones_sb = const.tile([P, P], f32, name="ones_sb")
            nc.sync.dma_start(ones_sb[:], ones_d[:])

            # ---- cast x (f32 -> bf16) into DRAM bounce ----
            xbf = dram.tile([L, C], bf16, name="xbf")
            for j in range(NCHUNK):
                nc.gpsimd.dma_start(xbf[ts(j, CHUNK), :], x_d[ts(j, CHUNK), :])

            # ---- transpose into [C, L] layout with halos ----
            xt = []
            for cb in range(NCB):
                t = xtp.tile([P, 2 * HALO + L], bf16, name=f"xt{cb}")
                xt.append(t)
                nc.gpsimd.memset(t[:, 0:HALO], 0.0)
                nc.gpsimd.memset(t[:, HALO + L:2 * HALO + L], 0.0)
            for cb in range(NCB):
                for j in range(NCHUNK):
                    nc.scalar.dma_start_transpose(
                        out=xt[cb][:, HALO + j * CHUNK: HALO + (j + 1) * CHUNK],
                        in_=xbf[ts(j, CHUNK), ts(cb, P)],
                    )

            # ---- conv + stats ----
            # conv[:, l] = w0*x[:, l-D] + w1*x[:, l] + w2*x[:, l+D]
            convt = [convp.tile([P, L], bf16, name=f"convt{cb}") for cb in range(NCB)]
            NK = NCB * NCHUNK
            stat_acc = const.tile([P, 2 * NK], f32, name="stat_acc")
            for cb in range(NCB):
                w0 = cw_sb[cb][:, 0:1]
                w1v = cw_sb[cb][:, 1:2]
                w2v = cw_sb[cb][:, 2:3]
                for j in range(NCHUNK):
                    lo = j * CHUNK
                    k = cb * NCHUNK + j
                    t2 = work.tile([P, CHUNK], bf16, name="t2", tag="t2")
                    # t2 = w2 * x[l+D]
                    nc.scalar.activation(
                        t2[:], xt[cb][:, lo + HALO + D: lo + HALO + D + CHUNK],
                        AF.Copy, bias=0.0, scale=w2v,
                    )
                    # s = w0 * x[l-D] + t2
                    s = work.tile([P, CHUNK], bf16, name="s", tag="s")
                    nc.vector.scalar_tensor_tensor(
                        s[:], xt[cb][:, lo + HALO - D: lo + HALO - D + CHUNK],
                        w0, t2[:], op0=OP.mult, op1=OP.add,
                    )
                    # conv = w1 * x[l] + s   (+ running sum into stat_acc[:, k])
                    nc.vector.scalar_tensor_tensor(
                        convt[cb][:, lo: lo + CHUNK],
                        xt[cb][:, lo + HALO: lo + HALO + CHUNK],
                        w1v, s[:], op0=OP.mult, op1=OP.add,
                        accum_out=stat_acc[:, k:k + 1],
                    )
                    # sum of conv^2 into stat_acc[:, NK + k]
                    sqj = work.tile([P, CHUNK], bf16, name="sqj", tag="sqj", bufs=1)
                    nc.scalar.activation(
                        sqj[:], convt[cb][:, lo: lo + CHUNK], AF.Square,
                        accum_out=stat_acc[:, NK + k: NK + k + 1],
                    )

            # ---- stats: cross-partition reduce via ones-matmul, then finalize ----
            stats_ps = psum.tile([P, 2 * NK], f32, name="stats_ps", tag="stats", bufs=1)
            nc.tensor.matmul(stats_ps[:], ones_sb[:], stat_acc[:], start=True, stop=True)
            tot = const.tile([P, 2], f32, name="tot")
            nc.vector.tensor_reduce(
                tot[:], stats_ps[:].rearrange("p (s k) -> p s k", s=2),
                axis=AX.X, op=OP.add,
            )
            inv_n = 1.0 / float(L * C)
            mean = const.tile([P, 1], f32, name="mean")
            nc.vector.tensor_scalar_mul(mean[:], tot[:, 0:1], inv_n)
            msq = const.tile([P, 1], f32, name="msq")
            nc.vector.tensor_scalar_mul(msq[:], tot[:, 1:2], inv_n)
            # nvar = mean^2 - E[conv^2]   (negated variance)
            nvar = const.tile([P, 1], f32, name="nvar")
            nc.vector.scalar_tensor_tensor(
                nvar[:], mean[:], mean[:, 0:1], msq[:], op0=OP.mult, op1=OP.subtract,
            )
            # sd = sqrt(eps - nvar) = sqrt(var + eps)
            epsb = const.tile([P, 1], f32, name="epsb")
            nc.gpsimd.memset(epsb[:], 1e-3)
            sd = const.tile([P, 1], f32, name="sd")
            nc.scalar.activation(sd[:], nvar[:], AF.Sqrt, bias=epsb[:, 0:1], scale=-1.0)
            rstd = const.tile([P, 1], f32, name="rstd")
            nc.vector.reciprocal(rstd[:], sd[:])
            # nmr = (-mean) * rstd
            nmr = const.tile([P, 1], f32, name="nmr")
            nc.vector.scalar_tensor_tensor(
                nmr[:], mean[:], -1.0, rstd[:], op0=OP.mult, op1=OP.mult,
            )
            # bias_all[:, hb] = -mean*rstd*s1[hb*128 + p]
            bias_all = const.tile([P, NHB], f32, name="bias_all")
            nc.vector.tensor_scalar_mul(bias_all[:], s1g_sb[:], nmr[:, 0:1])

            # ---- MM phase ----
            for i in range(NLT):
                l0 = i * LT
                hsb = []
                for hb in range(NHB):
                    ph = psum.tile([P, LT], f32, name="ph", tag="mm1", bufs=4)
                    for cb in range(NCB):
                        nc.tensor.matmul(
                            ph[:], w1t_sb[cb][:, ts(hb, P)], convt[cb][:, l0:l0 + LT],
                            start=(cb == 0), stop=(cb == NCB - 1),
                        )
                    ht = hp.tile([P, LT], bf16, name="ht", tag=f"h{hb}")
                    nc.scalar.activation(
                        ht[:], ph[:], AF.Gelu_apprx_tanh,
                        bias=bias_all[:, hb:hb + 1], scale=rstd[:, 0:1],
                    )
                    hsb.append(ht)
                for lsub in range(LT // P):
                    po = psum.tile([P, C], f32, name="po", tag="mm2", bufs=3)
                    for hb in range(NHB):
                        nc.tensor.matmul(
                            po[:], hsb[hb][:, ts(lsub, P)], w2tg_sb[hb][:],
                            start=(hb == 0), stop=(hb == NHB - 1),
                        )
                    row = l0 + lsub * P
                    xr = outp.tile([P, C], f32, name="xr", tag="xr")
                    nc.sync.dma_start(xr[:], x_d[row:row + P, :])
                    ot = outp.tile([P, C], f32, name="ot", tag="ot")
                    nc.vector.tensor_add(ot[:], po[:], xr[:])
                    nc.sync.dma_start(out_d[row:row + P, :], ot[:])

    nc.compile()
    return nc


def _get_module():
    if "nc" not in _CACHE:
        _CACHE["nc"] = _build_module()
    return _CACHE["nc"]


def kernel(X, conv_weight, W1, W2, gamma, dilation):
    import ml_dtypes
    from concourse.bass_utils import run_bass_kernel_spmd

    X = np.asarray(X, dtype=np.float32)
    conv_weight = np.asarray(conv_weight, dtype=np.float32)
    W1 = np.asarray(W1, dtype=np.float32)
    W2 = np.asarray(W2, dtype=np.float32)
    gamma = np.asarray(gamma, dtype=np.float32)

    N = X.shape[0]
    assert X.shape == (N_CORES, L, C) and int(dilation) == D
    assert W1.shape == (H, C) and W2.shape == (C, H)

    w1t = np.ascontiguousarray(W1.T).astype(ml_dtypes.bfloat16)            # [C, H]
    w2tg = np.ascontiguousarray((W2 * gamma.reshape(C, 1)).T).astype(
        ml_dtypes.bfloat16)                                                # [H, C]
    cwt = np.ascontiguousarray(conv_weight.T).astype(np.float32)           # [C, 3]
    s1 = W1.sum(axis=1).astype(np.float32)                                 # [H]
    s1g = np.ascontiguousarray(s1.reshape(NHB, P).T).astype(np.float32)    # [P, NHB]
    ones = np.ones((P, P), dtype=np.float32)

    nc = _get_module()
    in_maps = [
        {
            "x": np.ascontiguousarray(X[i]),
            "w1t": w1t,
            "w2tg": w2tg,
            "cwt": cwt,
            "s1g": s1g,
            "ones": ones,
        }
        for i in range(N_CORES)
    ]
    res = run_bass_kernel_spmd(nc, in_maps, core_ids=list(range(N_CORES)))
    out = np.stack([res.results[i]["out"] for i in range(N_CORES)], axis=0)
    return out.astype(np.float32)
